# revision 8
# baseline (speedup 1.0000x reference)
"""Causal self-attention on 8 TRN2 NeuronCores.

Problem: x[2,2048,1024], wq/wk/wv/wo[1024,1024] (nn.Linear convention,
out = y @ W.T), H=16 heads, D=64, causal softmax, f32 in/out.

Sharding: tensor-parallel over heads x data-parallel over batch.
Core i handles batch b=i//4 and head group g=i%4 (4 heads each).
Each core returns a partial output projection outT[b] and the host
sums the 4 partials per batch.

v3 design (evolved from traces of v1/v2):

- Everything the PE touches is bf16 (host-side cast): bf16 matmuls
  stream 1 column/cycle at any N, get fast weight loads, and let
  LDWEIGHTS overlap in-flight matmuls via the PE's reorder window.
- Scores for a head PAIR run concurrently in the PE array via row
  tiling: head 2p on SBUF partitions 0-63, head 2p+1 on 64-127, so
  the two 64-row stationaries occupy disjoint row groups and the two
  matmuls overlap to ~1x single-matmul duration.
- Attention is pair-outer, span-outer. One wide exp per
  (pair, span, key-chunk) covers both heads' score halves via a
  2-block strided AP over the 2-bank PSUM tile. The Act engine is the
  attention-phase bottleneck (~1 exp elem/cycle/partition), so the
  k/q projections for the second head pair are interleaved into
  pair0's attention loop two matmuls at a time, filling PE slack.
- Causal masking: multiplicative bf16 triangular mask on the diagonal
  128-strips after exp, on GpSimd (keeps DVE/Act queues clear).
- Softmax row sums ride a ones-column folded into V's stationary
  (65-wide PV matmuls). Normalization per (head, span): DVE copies PV
  out of PSUM fast, reciprocal_approx_fast on the sums row (~18-bit,
  plenty for a softmax denominator), one DMA roundtrip broadcasts
  1/sum across 64 partitions (stride-0 partition read), DVE multiply
  into the bf16 y tile.
- The output projection trails pair1's spans by one span; stores
  stream per 128-row block.
"""

import sys

for _p in ("/opt/trn_rl_repo", "/root/.axon_site"):
    if _p not in sys.path:
        sys.path.insert(0, _p)

import ml_dtypes
import numpy as np

import concourse.bass as bass
import concourse.mybir as mybir
import concourse.tile as tile
from concourse import bacc
from concourse.bass_utils import run_bass_kernel_spmd

B, T, C, H = 2, 2048, 1024, 16
DH = C // H            # 64 head dim
HG = 4                 # heads per core
GW = HG * DH           # 256 features per head group
NS = T // 512          # 4 query spans
KC = C // 128          # 8 contraction chunks over C
SCALE = 1.0 / float(np.sqrt(DH))
N_CORES = 8

F32 = mybir.dt.float32
BF16 = mybir.dt.bfloat16
EXP = mybir.ActivationFunctionType.Exp
COPY = mybir.ActivationFunctionType.Copy


def build_nc():
    nc = bacc.Bacc("TRN2", target_bir_lowering=False, debug=False,
                   num_devices=N_CORES)
    xT = nc.declare_dram_parameter("xT", [C, T], BF16, isOutput=False)
    wqT = nc.declare_dram_parameter("wqT", [C, GW], BF16, isOutput=False)
    wkT = nc.declare_dram_parameter("wkT", [C, GW], BF16, isOutput=False)
    wvT = nc.declare_dram_parameter("wvT", [C, GW], BF16, isOutput=False)
    woT = nc.declare_dram_parameter("woT", [GW, C], BF16, isOutput=False)
    outT = nc.declare_dram_parameter("outT", [C, T], F32, isOutput=True)
    # per (pair, head-in-pair, span): rowsum / 1/rowsum scratch for the
    # transpose + partition-broadcast roundtrips (reciprocal on a
    # [128,4] layout is ~20x cheaper than on [1,512])
    s_dram = nc.dram_tensor("s_scratch", [2, 2, NS, 512], F32)
    r_dram = nc.dram_tensor("r_scratch", [2, 2, NS, 512], F32)

    with tile.TileContext(nc) as tc:
        with tc.tile_pool(name="pers", bufs=1) as pers:
            # ---- persistent SBUF; DMAs issued in consumption order:
            # x span0 chunks first (first proj group needs all 8), then
            # wk, the x remainders, and the other weights ----
            xts = [pers.tile([128, T], BF16, tag=f"xT{i}", name=f"xT{i}")
                   for i in range(KC)]
            wk_t = pers.tile([128, KC * GW], BF16, tag="wk", name="wk")
            wq_t = pers.tile([128, KC * GW], BF16, tag="wq", name="wq")
            wv_t = pers.tile([128, KC * GW], BF16, tag="wv", name="wv")

            def wslice(wt, k, m=None):
                if m is None:
                    return wt[:, k * GW:(k + 1) * GW]
                return wt[:, k * GW + m * 128:k * GW + (m + 1) * 128]

            def wload(wt, src):
                # [128, KC*GW] <- src[128k+p, j] for chunk k, col j
                sap = src[:, :]
                nc.sync.dma_start(
                    out=wt.rearrange("p (k j) -> p k j", j=GW),
                    in_=bass.AP(tensor=sap.tensor, offset=sap.offset,
                                ap=[[GW, 128], [128 * GW, KC], [1, GW]]))

            for i in range(KC):
                nc.sync.dma_start(out=xts[i][:, 0:512],
                                  in_=xT[i * 128:(i + 1) * 128, 0:512])
            wload(wk_t, wkT)
            for i in range(KC):
                nc.sync.dma_start(out=xts[i][:, 512:T],
                                  in_=xT[i * 128:(i + 1) * 128, 512:T])
            wload(wq_t, wqT)
            wload(wv_t, wvT)
            wo_t = [pers.tile([128, C], BF16, tag=f"wo{j}", name=f"wo{j}")
                    for j in range(2)]
            for j in range(2):
                nc.sync.dma_start(out=wo_t[j], in_=woT[j * 128:(j + 1) * 128, :])

            # proj outputs / attention outputs, bf16, feature-major
            qts = [pers.tile([128, T], BF16, tag=f"qT{m}", name=f"qT{m}")
                   for m in range(2)]
            kts = [pers.tile([128, T], BF16, tag=f"kT{m}", name=f"kT{m}")
                   for m in range(2)]
            yts = [pers.tile([128, T], BF16, tag=f"yT{m}", name=f"yT{m}")
                   for m in range(2)]

            # bf16 triangular mask for the diagonal 128x128 strip of
            # P^T: keep (1) where col >= row i.e. q >= k, else 0
            trim = pers.tile([128, 128], BF16, tag="trim", name="trim")
            nc.gpsimd.memset(trim, 1.0)
            nc.gpsimd.affine_select(
                out=trim, in_=trim, compare_op=mybir.AluOpType.is_ge,
                fill=0.0, base=0, pattern=[[1, 128]], channel_multiplier=-1)
            # ones [128, 4] in bf16 for V's ones-columns
            ones4 = pers.tile([128, 4], BF16, tag="ones4", name="ones4")
            for j in range(4):
                nc.scalar.activation(
                    out=ones4[:, j:j + 1],
                    in_=nc.const_aps.tensor(1.0, [128, 1]), func=COPY)

            # V in natural [t, d] layout, 65-wide per head (64 v + one)
            vts = [pers.tile([128, HG * 65], BF16, tag=f"V{tb}", name=f"V{tb}")
                   for tb in range(T // 128)]

            # ---- phase 1: k/q projections for pair0 + all of V ----
            with tc.tile_pool(name="pp1", bufs=4, space="PSUM") as pp1, \
                 tc.tile_pool(name="pp2", bufs=2, space="PSUM") as pp2:
                def proj_block(wt, dest, m, pool):
                    for s in range(NS):
                        ps = pool.tile([128, 512], F32, tag="projps",
                                       name="projps")
                        for k in range(KC):
                            nc.tensor.matmul(
                                ps,
                                wslice(wt, k, m),
                                xts[k][:, s * 512:(s + 1) * 512],
                                start=(k == 0), stop=(k == KC - 1))
                        nc.vector.tensor_copy(
                            out=dest[m][:, s * 512:(s + 1) * 512], in_=ps)

                proj_block(wk_t, kts, 0, pp1)
                proj_block(wq_t, qts, 0, pp1)
                for tb in range(T // 128):
                    vps = pp2.tile([128, GW], F32, tag="vps", name="vps")
                    for k in range(KC):
                        nc.tensor.matmul(
                            vps, xts[k][:, tb * 128:(tb + 1) * 128],
                            wslice(wv_t, k),
                            start=(k == 0), stop=(k == KC - 1))
                    vt = vts[tb]
                    nc.vector.tensor_copy(
                        out=vt.rearrange("p (h c) -> p h c", c=65)[:, :, 0:64],
                        in_=vps.rearrange("p (h c) -> p h c", c=64))
                    nc.vector.tensor_copy(
                        out=vt.rearrange("p (h c) -> p h c", c=65)[:, :, 64],
                        in_=ones4)

            # ---- phase 2: attention (pair-outer), k/q-m1 projections
            # interleaved into pair0, out-projection trailing pair1 ----
            with tc.tile_pool(name="mgs", bufs=2, space="PSUM") as mgs, \
                 tc.tile_pool(name="pvs", bufs=1, space="PSUM") as pvs, \
                 tc.tile_pool(name="ops", bufs=2, space="PSUM") as ops, \
                 tc.tile_pool(name="ptp", bufs=6) as ptp, \
                 tc.tile_pool(name="rp", bufs=4) as rp, \
                 tc.tile_pool(name="ost", bufs=4) as ost:

                # deferred k/q m=1 projection, dribbled 2 matmuls at a
                # time into pair0's attention loop (fills PE slack
                # while Act streams exps)
                proj_q = []       # list of (psum_tile_or_None-marker)
                state = {"ps": None}

                def proj_steps():
                    for wt, dest in ((wk_t, kts), (wq_t, qts)):
                        for s in range(NS):
                            for k in range(KC):
                                yield (wt, dest, s, k)

                proj_iter = proj_steps()

                def emit_proj(n):
                    for _ in range(n):
                        step = next(proj_iter, None)
                        if step is None:
                            return
                        wt, dest, s, k = step
                        if k == 0:
                            state["ps"] = ops.tile([128, 512], F32,
                                                   tag="op", name="op")
                        nc.tensor.matmul(
                            state["ps"], wslice(wt, k, 1),
                            xts[k][:, s * 512:(s + 1) * 512],
                            start=(k == 0), stop=(k == KC - 1))
                        if k == KC - 1:
                            nc.vector.tensor_copy(
                                out=dest[1][:, s * 512:(s + 1) * 512],
                                in_=state["ps"])

                def outproj(s):
                    for m in range(8):
                        op = ops.tile([128, 512], F32, tag="op", name="op")
                        for j in range(2):
                            nc.tensor.matmul(
                                op,
                                wo_t[j][:, m * 128:(m + 1) * 128],
                                yts[j][:, s * 512:(s + 1) * 512],
                                start=(j == 0), stop=(j == 1))
                        ot = ost.tile([128, 512], F32, tag="ot", name="ot")
                        nc.vector.tensor_copy(out=ot, in_=op)
                        nc.sync.dma_start(
                            out=outT[m * 128:(m + 1) * 128,
                                     s * 512:(s + 1) * 512],
                            in_=ot)

                for p in range(2):
                    qt, kt, yt = qts[p], kts[p], yts[p]
                    for s in range(NS):
                        pv = [pvs.tile([65, 512], F32, tag=f"pv{hl}",
                                       name=f"pv{hl}") for hl in range(2)]
                        for ki in range(4 * s + 4):
                            c0 = 128 * (ki - 4 * s) if ki >= 4 * s else 0
                            w = 512 - c0
                            q0 = s * 512 + c0
                            mg = mgs.tile([128, 1024], F32, tag="mg",
                                          name="mg")
                            # paired scores: head 2p rows 0-63 ->
                            # bank A, head 2p+1 rows 64-127 -> bank B;
                            # concurrent via row tiling
                            nc.tensor.matmul(
                                mg[:, c0:512],
                                kt[0:64, ki * 128:(ki + 1) * 128],
                                qt[0:64, q0:(s + 1) * 512],
                                start=True, stop=True)
                            nc.tensor.matmul(
                                mg[:, 512 + c0:1024],
                                kt[64:128, ki * 128:(ki + 1) * 128],
                                qt[64:128, q0:(s + 1) * 512],
                                start=True, stop=True)
                            # one exp over both heads' halves
                            pt = ptp.tile([128, 1024], BF16, tag="pt",
                                          name="pt")
                            mga = bass.AP(
                                tensor=mg.tensor, offset=mg.offset + c0,
                                ap=[list(mg.ap[0]), [512, 2], [1, w]])
                            pta = bass.AP(
                                tensor=pt.tensor, offset=pt.offset + c0,
                                ap=[list(pt.ap[0]), [512, 2], [1, w]])
                            nc.scalar.activation(
                                out=pta, in_=mga, func=EXP, scale=SCALE)
                            if ki >= 4 * s:
                                # causal mask on the diagonal strips
                                nc.gpsimd.tensor_mul(
                                    out=pt[:, c0:c0 + 128],
                                    in0=pt[:, c0:c0 + 128], in1=trim)
                                nc.gpsimd.tensor_mul(
                                    out=pt[:, 512 + c0:512 + c0 + 128],
                                    in0=pt[:, 512 + c0:512 + c0 + 128],
                                    in1=trim)
                            for hl in range(2):
                                h = 2 * p + hl
                                nc.tensor.matmul(
                                    pv[hl][:, c0:512],
                                    vts[ki][:, h * 65:(h + 1) * 65],
                                    pt[:, 512 * hl + c0:512 * hl + c0 + w],
                                    start=(ki == 0), stop=(ki == 4 * s + 3))
                            if p == 0:
                                emit_proj(2)
                        # finalize both heads of (pair p, span s)
                        for hl in range(2):
                            po = hl * 64
                            yv = rp.tile([65, 512], F32, tag=f"yv{hl}",
                                         name=f"yv{hl}")
                            nc.vector.tensor_copy(out=yv, in_=pv[hl][0:65, :])
                            nc.gpsimd.dma_start(out=s_dram[p, hl, s, :],
                                                in_=yv[64:65, :])
                            st = rp.tile([128, 4], F32, tag=f"st{hl}",
                                         name=f"st{hl}")
                            nc.gpsimd.dma_start(
                                out=st,
                                in_=s_dram[p, hl, s, :].rearrange(
                                    "(c p) -> p c", p=128))
                            rts = rp.tile([128, 4], F32, tag=f"rts{hl}",
                                         name=f"rts{hl}")
                            nc.vector.reciprocal(out=rts, in_=st)
                            nc.gpsimd.dma_start(
                                out=r_dram[p, hl, s, :].rearrange(
                                    "(c p) -> p c", p=128),
                                in_=rts)
                            rb = rp.tile([64, 512], F32, tag=f"rb{hl}",
                                         name=f"rb{hl}")
                            rsl = r_dram[p, hl, s, :]
                            nc.sync.dma_start(
                                out=rb,
                                in_=bass.AP(tensor=rsl.tensor,
                                            offset=rsl.offset,
                                            ap=[[0, 64]] + list(rsl.ap)))
                            nc.vector.tensor_mul(
                                out=yt[po:po + 64, s * 512:(s + 1) * 512],
                                in0=yv[0:64, :], in1=rb)
                        if p == 1 and s > 0:
                            outproj(s - 1)
                    if p == 0:
                        emit_proj(64)   # drain any leftovers
                outproj(NS - 1)
    nc.compile()
    return nc


_NC_CACHE = None


def _get_nc():
    global _NC_CACHE
    if _NC_CACHE is None:
        _NC_CACHE = build_nc()
    return _NC_CACHE


def make_in_maps(x, wq, wk, wv, wo):
    bf = ml_dtypes.bfloat16
    x = np.asarray(x, dtype=np.float32)
    wq = np.asarray(wq, dtype=np.float32)
    wk = np.asarray(wk, dtype=np.float32)
    wv = np.asarray(wv, dtype=np.float32)
    wo = np.asarray(wo, dtype=np.float32)
    in_maps = []
    for core in range(N_CORES):
        b, g = core // HG, core % HG
        rows = slice(g * GW, (g + 1) * GW)
        in_maps.append({
            "xT": np.ascontiguousarray(x[b].T).astype(bf),
            "wqT": np.ascontiguousarray(wq[rows, :].T).astype(bf),
            "wkT": np.ascontiguousarray(wk[rows, :].T).astype(bf),
            "wvT": np.ascontiguousarray(wv[rows, :].T).astype(bf),
            "woT": np.ascontiguousarray(wo[:, rows].T).astype(bf),
        })
    return in_maps


def run(x, wq, wk, wv, wo, trace=False, tmpdir=None):
    nc = _get_nc()
    in_maps = make_in_maps(x, wq, wk, wv, wo)
    res = run_bass_kernel_spmd(nc, in_maps, core_ids=list(range(N_CORES)),
                               trace=trace, tmpdir=tmpdir)
    out = np.zeros((B, T, C), dtype=np.float32)
    for core in range(N_CORES):
        out[core // HG] += res.results[core]["outT"].T
    return out, res


def kernel(x, wq, wk, wv, wo):
    out, _ = run(x, wq, wk, wv, wo)
    return out


# revision 11
# speedup vs baseline: 1.0213x; 1.0213x over previous
"""Causal self-attention on 8 TRN2 NeuronCores.

Problem: x[2,2048,1024], wq/wk/wv/wo[1024,1024] (nn.Linear convention,
out = y @ W.T), H=16 heads, D=64, causal softmax, f32 in/out.

Sharding: tensor-parallel over heads x data-parallel over batch.
Core i handles batch b=i//4 and head group g=i%4 (4 heads each).
Each core returns a partial output projection outT[b] and the host
sums the 4 partials per batch.

v3 design (evolved from traces of v1/v2):

- Everything the PE touches is bf16 (host-side cast): bf16 matmuls
  stream 1 column/cycle at any N, get fast weight loads, and let
  LDWEIGHTS overlap in-flight matmuls via the PE's reorder window.
- Scores for a head PAIR run concurrently in the PE array via row
  tiling: head 2p on SBUF partitions 0-63, head 2p+1 on 64-127, so
  the two 64-row stationaries occupy disjoint row groups and the two
  matmuls overlap to ~1x single-matmul duration.
- Attention is pair-outer, span-outer. One wide exp per
  (pair, span, key-chunk) covers both heads' score halves via a
  2-block strided AP over the 2-bank PSUM tile. The Act engine is the
  attention-phase bottleneck (~1 exp elem/cycle/partition), so the
  k/q projections for the second head pair are interleaved into
  pair0's attention loop two matmuls at a time, filling PE slack.
- Causal masking: multiplicative bf16 triangular mask on the diagonal
  128-strips after exp, on GpSimd (keeps DVE/Act queues clear).
- Softmax row sums ride a ones-column folded into V's stationary
  (65-wide PV matmuls). Normalization per (head, span): DVE copies PV
  out of PSUM fast, reciprocal_approx_fast on the sums row (~18-bit,
  plenty for a softmax denominator), one DMA roundtrip broadcasts
  1/sum across 64 partitions (stride-0 partition read), DVE multiply
  into the bf16 y tile.
- The output projection trails pair1's spans by one span; stores
  stream per 128-row block.
"""

import sys

for _p in ("/opt/trn_rl_repo", "/root/.axon_site"):
    if _p not in sys.path:
        sys.path.insert(0, _p)

import ml_dtypes
import numpy as np

import concourse.bass as bass
import concourse.mybir as mybir
import concourse.tile as tile
from concourse import bacc
from concourse.bass_utils import run_bass_kernel_spmd

B, T, C, H = 2, 2048, 1024, 16
DH = C // H            # 64 head dim
HG = 4                 # heads per core
GW = HG * DH           # 256 features per head group
NS = T // 512          # 4 query spans
KC = C // 128          # 8 contraction chunks over C
SCALE = 1.0 / float(np.sqrt(DH))
N_CORES = 8

F32 = mybir.dt.float32
BF16 = mybir.dt.bfloat16
EXP = mybir.ActivationFunctionType.Exp
COPY = mybir.ActivationFunctionType.Copy


def build_nc():
    nc = bacc.Bacc("TRN2", target_bir_lowering=False, debug=False,
                   num_devices=N_CORES)
    xT = nc.declare_dram_parameter("xT", [C, T], BF16, isOutput=False)
    wqT = nc.declare_dram_parameter("wqT", [C, GW], BF16, isOutput=False)
    wkT = nc.declare_dram_parameter("wkT", [C, GW], BF16, isOutput=False)
    wvT = nc.declare_dram_parameter("wvT", [C, GW], BF16, isOutput=False)
    woT = nc.declare_dram_parameter("woT", [GW, C], BF16, isOutput=False)
    outT = nc.declare_dram_parameter("outT", [C, T], F32, isOutput=True)
    # per (pair, head-in-pair, span): rowsum / 1/rowsum scratch for the
    # transpose + partition-broadcast roundtrips (reciprocal on a
    # [128,4] layout is ~20x cheaper than on [1,512])
    s_dram = nc.dram_tensor("s_scratch", [2, 2, NS, 512], F32)
    r_dram = nc.dram_tensor("r_scratch", [2, 2, NS, 512], F32)

    with tile.TileContext(nc) as tc:
        with tc.tile_pool(name="pers", bufs=1) as pers:
            # ---- persistent SBUF; DMAs issued in consumption order:
            # x span0 chunks first (first proj group needs all 8), then
            # wk, the x remainders, and the other weights ----
            xts = [pers.tile([128, T], BF16, tag=f"xT{i}", name=f"xT{i}")
                   for i in range(KC)]
            wk_t = pers.tile([128, KC * GW], BF16, tag="wk", name="wk")
            wq_t = pers.tile([128, KC * GW], BF16, tag="wq", name="wq")
            wv_t = pers.tile([128, KC * GW], BF16, tag="wv", name="wv")

            def wslice(wt, k, m=None):
                if m is None:
                    return wt[:, k * GW:(k + 1) * GW]
                return wt[:, k * GW + m * 128:k * GW + (m + 1) * 128]

            def wload(wt, src):
                # [128, KC*GW] <- src[128k+p, j] for chunk k, col j
                sap = src[:, :]
                nc.sync.dma_start(
                    out=wt.rearrange("p (k j) -> p k j", j=GW),
                    in_=bass.AP(tensor=sap.tensor, offset=sap.offset,
                                ap=[[GW, 128], [128 * GW, KC], [1, GW]]))

            for i in range(KC):
                nc.sync.dma_start(out=xts[i][:, 0:512],
                                  in_=xT[i * 128:(i + 1) * 128, 0:512])
            wload(wk_t, wkT)
            for i in range(KC):
                nc.sync.dma_start(out=xts[i][:, 512:T],
                                  in_=xT[i * 128:(i + 1) * 128, 512:T])
            wload(wq_t, wqT)
            wload(wv_t, wvT)
            wo_t = [pers.tile([128, C], BF16, tag=f"wo{j}", name=f"wo{j}")
                    for j in range(2)]
            for j in range(2):
                nc.sync.dma_start(out=wo_t[j], in_=woT[j * 128:(j + 1) * 128, :])

            # proj outputs / attention outputs, bf16, feature-major
            qts = [pers.tile([128, T], BF16, tag=f"qT{m}", name=f"qT{m}")
                   for m in range(2)]
            kts = [pers.tile([128, T], BF16, tag=f"kT{m}", name=f"kT{m}")
                   for m in range(2)]
            yts = [pers.tile([128, T], BF16, tag=f"yT{m}", name=f"yT{m}")
                   for m in range(2)]

            # bf16 triangular mask for the diagonal 128x128 strip of
            # P^T: keep (1) where col >= row i.e. q >= k, else 0
            trim = pers.tile([128, 128], BF16, tag="trim", name="trim")
            nc.gpsimd.memset(trim, 1.0)
            nc.gpsimd.affine_select(
                out=trim, in_=trim, compare_op=mybir.AluOpType.is_ge,
                fill=0.0, base=0, pattern=[[1, 128]], channel_multiplier=-1)
            # ones [128, 4] in bf16 for V's ones-columns
            ones4 = pers.tile([128, 4], BF16, tag="ones4", name="ones4")
            for j in range(4):
                nc.scalar.activation(
                    out=ones4[:, j:j + 1],
                    in_=nc.const_aps.tensor(1.0, [128, 1]), func=COPY)

            # V in natural [t, d] layout, 65-wide per head (64 v + one)
            vts = [pers.tile([128, HG * 65], BF16, tag=f"V{tb}", name=f"V{tb}")
                   for tb in range(T // 128)]

            # ---- phase 1: k/q projections for pair0 + all of V ----
            with tc.tile_pool(name="pp1", bufs=4, space="PSUM") as pp1, \
                 tc.tile_pool(name="pp2", bufs=2, space="PSUM") as pp2:
                def proj_block(wt, dest, m, pool):
                    for s in range(NS):
                        ps = pool.tile([128, 512], F32, tag="projps",
                                       name="projps")
                        for k in range(KC):
                            nc.tensor.matmul(
                                ps,
                                wslice(wt, k, m),
                                xts[k][:, s * 512:(s + 1) * 512],
                                start=(k == 0), stop=(k == KC - 1))
                        nc.vector.tensor_copy(
                            out=dest[m][:, s * 512:(s + 1) * 512], in_=ps)

                proj_block(wk_t, kts, 0, pp1)
                proj_block(wq_t, qts, 0, pp1)
                for tb in range(T // 128):
                    vps = pp2.tile([128, GW], F32, tag="vps", name="vps")
                    for k in range(KC):
                        nc.tensor.matmul(
                            vps, xts[k][:, tb * 128:(tb + 1) * 128],
                            wslice(wv_t, k),
                            start=(k == 0), stop=(k == KC - 1))
                    vt = vts[tb]
                    nc.vector.tensor_copy(
                        out=vt.rearrange("p (h c) -> p h c", c=65)[:, :, 0:64],
                        in_=vps.rearrange("p (h c) -> p h c", c=64))
                    nc.vector.tensor_copy(
                        out=vt.rearrange("p (h c) -> p h c", c=65)[:, :, 64],
                        in_=ones4)

            # ---- phase 2: attention (pair-outer), k/q-m1 projections
            # interleaved into pair0, out-projection trailing pair1 ----
            with tc.tile_pool(name="mgs", bufs=2, space="PSUM") as mgs, \
                 tc.tile_pool(name="pvs", bufs=1, space="PSUM") as pvs, \
                 tc.tile_pool(name="ops", bufs=2, space="PSUM") as ops, \
                 tc.tile_pool(name="ptp", bufs=6) as ptp, \
                 tc.tile_pool(name="rp", bufs=4) as rp, \
                 tc.tile_pool(name="ost", bufs=4) as ost:

                # deferred k/q m=1 projection, dribbled 2 matmuls at a
                # time into pair0's attention loop (fills PE slack
                # while Act streams exps)
                proj_q = []       # list of (psum_tile_or_None-marker)
                state = {"ps": None}

                def proj_steps():
                    for wt, dest in ((wk_t, kts), (wq_t, qts)):
                        for s in range(NS):
                            for k in range(KC):
                                yield (wt, dest, s, k)

                proj_iter = proj_steps()

                def emit_proj(n):
                    for _ in range(n):
                        step = next(proj_iter, None)
                        if step is None:
                            return
                        wt, dest, s, k = step
                        if k == 0:
                            state["ps"] = ops.tile([128, 512], F32,
                                                   tag="op", name="op")
                        nc.tensor.matmul(
                            state["ps"], wslice(wt, k, 1),
                            xts[k][:, s * 512:(s + 1) * 512],
                            start=(k == 0), stop=(k == KC - 1))
                        if k == KC - 1:
                            nc.vector.tensor_copy(
                                out=dest[1][:, s * 512:(s + 1) * 512],
                                in_=state["ps"])

                def outproj(s):
                    for m in range(8):
                        op = ops.tile([128, 512], F32, tag="op", name="op")
                        for j in range(2):
                            nc.tensor.matmul(
                                op,
                                wo_t[j][:, m * 128:(m + 1) * 128],
                                yts[j][:, s * 512:(s + 1) * 512],
                                start=(j == 0), stop=(j == 1))
                        ot = ost.tile([128, 512], F32, tag="ot", name="ot")
                        nc.vector.tensor_copy(out=ot, in_=op)
                        nc.sync.dma_start(
                            out=outT[m * 128:(m + 1) * 128,
                                     s * 512:(s + 1) * 512],
                            in_=ot)

                # finalize chains are staged so every op's wait is
                # pre-satisfied when its queue reaches it:
                #   A  (own span end):   yv copy (frees PSUM), sums->DRAM
                #   B1 (next span, after 1st ki): transposed ld, recip,
                #                        recip->DRAM, broadcast ld
                #   B2 (next span end):  normalize muls (before outproj)
                pend_b1 = []
                pend_b2 = []
                yv_map = {}

                def fin_a(p, s):
                    out = []
                    for hl in range(2):
                        yv = rp.tile([65, 512], F32, tag=f"yv{p}{hl}",
                                     name=f"yv{p}{hl}")
                        nc.vector.tensor_copy(out=yv, in_=pv_cur[hl][0:65, :])
                        nc.sync.dma_start(out=s_dram[p, hl, s, :],
                                          in_=yv[64:65, :])
                        out.append(yv)
                    return out

                def fin_b1(p, s):
                    out = []
                    for hl in range(2):
                        st = rp.tile([128, 4], F32, tag=f"st{hl}",
                                     name=f"st{hl}")
                        nc.sync.dma_start(
                            out=st,
                            in_=s_dram[p, hl, s, :].rearrange(
                                "(c p) -> p c", p=128))
                        rts = rp.tile([128, 4], F32, tag=f"rts{hl}",
                                      name=f"rts{hl}")
                        nc.vector.reciprocal(out=rts, in_=st)
                        nc.sync.dma_start(
                            out=r_dram[p, hl, s, :].rearrange(
                                "(c p) -> p c", p=128),
                            in_=rts)
                        rb = rp.tile([64, 512], F32, tag=f"rb{p}{hl}",
                                     name=f"rb{p}{hl}")
                        rsl = r_dram[p, hl, s, :]
                        nc.sync.dma_start(
                            out=rb,
                            in_=bass.AP(tensor=rsl.tensor, offset=rsl.offset,
                                        ap=[[0, 64]] + list(rsl.ap)))
                        out.append(rb)
                    return out

                def fin_b2(p, s, yvs, rbs):
                    for hl in range(2):
                        po = hl * 64
                        nc.vector.tensor_mul(
                            out=yts[p][po:po + 64, s * 512:(s + 1) * 512],
                            in0=yvs[hl][0:64, :], in1=rbs[hl])

                for p in range(2):
                    qt, kt = qts[p], kts[p]
                    for s in range(NS):
                        pv_cur = [pvs.tile([65, 512], F32, tag=f"pv{hl}",
                                           name=f"pv{hl}") for hl in range(2)]
                        for ki in range(4 * s + 4):
                            c0 = 128 * (ki - 4 * s) if ki >= 4 * s else 0
                            w = 512 - c0
                            q0 = s * 512 + c0
                            mg = mgs.tile([128, 1024], F32, tag="mg",
                                          name="mg")
                            # paired scores: head 2p rows 0-63 ->
                            # bank A, head 2p+1 rows 64-127 -> bank B;
                            # concurrent via row tiling
                            nc.tensor.matmul(
                                mg[:, c0:512],
                                kt[0:64, ki * 128:(ki + 1) * 128],
                                qt[0:64, q0:(s + 1) * 512],
                                start=True, stop=True)
                            nc.tensor.matmul(
                                mg[:, 512 + c0:1024],
                                kt[64:128, ki * 128:(ki + 1) * 128],
                                qt[64:128, q0:(s + 1) * 512],
                                start=True, stop=True)
                            # one exp over both heads' halves
                            pt = ptp.tile([128, 1024], BF16, tag="pt",
                                          name="pt")
                            mga = bass.AP(
                                tensor=mg.tensor, offset=mg.offset + c0,
                                ap=[list(mg.ap[0]), [512, 2], [1, w]])
                            pta = bass.AP(
                                tensor=pt.tensor, offset=pt.offset + c0,
                                ap=[list(pt.ap[0]), [512, 2], [1, w]])
                            nc.scalar.activation(
                                out=pta, in_=mga, func=EXP, scale=SCALE)
                            if ki >= 4 * s:
                                # causal mask on the diagonal strips
                                # (gpsimd queue carries ONLY these)
                                nc.gpsimd.tensor_mul(
                                    out=pt[:, c0:c0 + 128],
                                    in0=pt[:, c0:c0 + 128], in1=trim)
                                nc.gpsimd.tensor_mul(
                                    out=pt[:, 512 + c0:512 + c0 + 128],
                                    in0=pt[:, 512 + c0:512 + c0 + 128],
                                    in1=trim)
                            for hl in range(2):
                                h = 2 * p + hl
                                nc.tensor.matmul(
                                    pv_cur[hl][:, c0:512],
                                    vts[ki][:, h * 65:(h + 1) * 65],
                                    pt[:, 512 * hl + c0:512 * hl + c0 + w],
                                    start=(ki == 0), stop=(ki == 4 * s + 3))
                            if p == 0:
                                emit_proj(2)
                            if ki == 0:
                                # B1 of the previous span's chains
                                for (pp, ss) in pend_b1:
                                    rbs = fin_b1(pp, ss)
                                    pend_b2.append((pp, ss, rbs))
                                pend_b1.clear()
                        # span end: B2 of the previous chains, then this
                        # span's A, then the trailing out-projection
                        for (pp, ss, rbs) in pend_b2:
                            fin_b2(pp, ss, yv_map[(pp, ss)], rbs)
                        pend_b2.clear()
                        yv_map[(p, s)] = fin_a(p, s)
                        pend_b1.append((p, s))
                        if p == 1 and s > 0:
                            outproj(s - 1)
                    if p == 0:
                        emit_proj(64)   # drain any leftovers
                # tail: flush the last span's chain, then its outproj
                for (pp, ss) in pend_b1:
                    rbs = fin_b1(pp, ss)
                    pend_b2.append((pp, ss, rbs))
                pend_b1.clear()
                for (pp, ss, rbs) in pend_b2:
                    fin_b2(pp, ss, yv_map[(pp, ss)], rbs)
                pend_b2.clear()
                outproj(NS - 1)
    nc.compile()
    return nc


_NC_CACHE = None


def _get_nc():
    global _NC_CACHE
    if _NC_CACHE is None:
        _NC_CACHE = build_nc()
    return _NC_CACHE


def make_in_maps(x, wq, wk, wv, wo):
    bf = ml_dtypes.bfloat16
    x = np.asarray(x, dtype=np.float32)
    wq = np.asarray(wq, dtype=np.float32)
    wk = np.asarray(wk, dtype=np.float32)
    wv = np.asarray(wv, dtype=np.float32)
    wo = np.asarray(wo, dtype=np.float32)
    in_maps = []
    for core in range(N_CORES):
        b, g = core // HG, core % HG
        rows = slice(g * GW, (g + 1) * GW)
        in_maps.append({
            "xT": np.ascontiguousarray(x[b].T).astype(bf),
            "wqT": np.ascontiguousarray(wq[rows, :].T).astype(bf),
            "wkT": np.ascontiguousarray(wk[rows, :].T).astype(bf),
            "wvT": np.ascontiguousarray(wv[rows, :].T).astype(bf),
            "woT": np.ascontiguousarray(wo[:, rows].T).astype(bf),
        })
    return in_maps


def run(x, wq, wk, wv, wo, trace=False, tmpdir=None):
    nc = _get_nc()
    in_maps = make_in_maps(x, wq, wk, wv, wo)
    res = run_bass_kernel_spmd(nc, in_maps, core_ids=list(range(N_CORES)),
                               trace=trace, tmpdir=tmpdir)
    out = np.zeros((B, T, C), dtype=np.float32)
    for core in range(N_CORES):
        out[core // HG] += res.results[core]["outT"].T
    return out, res


def kernel(x, wq, wk, wv, wo):
    out, _ = run(x, wq, wk, wv, wo)
    return out


# revision 16
# speedup vs baseline: 1.5048x; 1.4734x over previous
"""Causal self-attention on 8 TRN2 NeuronCores.

Problem: x[2,2048,1024], wq/wk/wv/wo[1024,1024] (nn.Linear convention,
out = y @ W.T), H=16 heads, D=64, causal softmax, f32 in/out.

Sharding: tensor-parallel over heads x data-parallel over batch.
Core i handles batch b=i//4 and head group g=i%4 (4 heads each).
Each core returns a partial output projection outT[b] and the host
sums the 4 partials per batch.

v3 design (evolved from traces of v1/v2):

- Everything the PE touches is bf16 (host-side cast): bf16 matmuls
  stream 1 column/cycle at any N, get fast weight loads, and let
  LDWEIGHTS overlap in-flight matmuls via the PE's reorder window.
- Scores for a head PAIR run concurrently in the PE array via row
  tiling: head 2p on SBUF partitions 0-63, head 2p+1 on 64-127, so
  the two 64-row stationaries occupy disjoint row groups and the two
  matmuls overlap to ~1x single-matmul duration.
- Attention is pair-outer, span-outer. One wide exp per
  (pair, span, key-chunk) covers both heads' score halves via a
  2-block strided AP over the 2-bank PSUM tile. The Act engine is the
  attention-phase bottleneck (~1 exp elem/cycle/partition), so the
  k/q projections for the second head pair are interleaved into
  pair0's attention loop two matmuls at a time, filling PE slack.
- Causal masking: multiplicative bf16 triangular mask on the diagonal
  128-strips after exp, on GpSimd (keeps DVE/Act queues clear).
- Softmax row sums ride a ones-column folded into V's stationary
  (65-wide PV matmuls). Normalization per (head, span): DVE copies PV
  out of PSUM fast, reciprocal_approx_fast on the sums row (~18-bit,
  plenty for a softmax denominator), one DMA roundtrip broadcasts
  1/sum across 64 partitions (stride-0 partition read), DVE multiply
  into the bf16 y tile.
- The output projection trails pair1's spans by one span; stores
  stream per 128-row block.
"""

import sys

for _p in ("/opt/trn_rl_repo", "/root/.axon_site"):
    if _p not in sys.path:
        sys.path.insert(0, _p)

import ml_dtypes
import numpy as np

import concourse.bass as bass
import concourse.mybir as mybir
import concourse.tile as tile
from concourse import bacc
from concourse.bass_utils import run_bass_kernel_spmd

B, T, C, H = 2, 2048, 1024, 16
DH = C // H            # 64 head dim
HG = 4                 # heads per core
GW = HG * DH           # 256 features per head group
NS = T // 512          # 4 query spans
KC = C // 128          # 8 contraction chunks over C
SCALE = 1.0 / float(np.sqrt(DH))
N_CORES = 8

F32 = mybir.dt.float32
BF16 = mybir.dt.bfloat16
EXP = mybir.ActivationFunctionType.Exp
COPY = mybir.ActivationFunctionType.Copy


def build_nc():
    nc = bacc.Bacc("TRN2", target_bir_lowering=False, debug=False,
                   num_devices=N_CORES)
    xT = nc.declare_dram_parameter("xT", [C, T], BF16, isOutput=False)
    wqT = nc.declare_dram_parameter("wqT", [C, GW], BF16, isOutput=False)
    wkT = nc.declare_dram_parameter("wkT", [C, GW], BF16, isOutput=False)
    wvT = nc.declare_dram_parameter("wvT", [C, GW], BF16, isOutput=False)
    woT = nc.declare_dram_parameter("woT", [GW, C], BF16, isOutput=False)
    outT = nc.declare_dram_parameter("outT", [C, T], F32, isOutput=True)
    # per (pair, head-in-pair, span): rowsum / 1/rowsum scratch for the
    # transpose + partition-broadcast roundtrips (reciprocal on a
    # [128,4] layout is ~20x cheaper than on [1,512])
    s_dram = nc.dram_tensor("s_scratch", [2, 2, NS, 512], F32)
    r_dram = nc.dram_tensor("r_scratch", [2, 2, NS, 512], F32)

    with tile.TileContext(nc) as tc:
        with tc.tile_pool(name="pers", bufs=1) as pers:
            # ---- persistent SBUF; DMAs issued in consumption order:
            # x span0 chunks first (first proj group needs all 8), then
            # wk, the x remainders, and the other weights ----
            xts = [pers.tile([128, T], BF16, tag=f"xT{i}", name=f"xT{i}")
                   for i in range(KC)]
            wk_t = pers.tile([128, KC * GW], BF16, tag="wk", name="wk")
            wq_t = pers.tile([128, KC * GW], BF16, tag="wq", name="wq")
            wv_t = pers.tile([128, KC * GW], BF16, tag="wv", name="wv")

            def wslice(wt, k, m=None):
                if m is None:
                    return wt[:, k * GW:(k + 1) * GW]
                return wt[:, k * GW + m * 128:k * GW + (m + 1) * 128]

            def wload(wt, src):
                # [128, KC*GW] <- src[128k+p, j] for chunk k, col j
                sap = src[:, :]
                nc.sync.dma_start(
                    out=wt.rearrange("p (k j) -> p k j", j=GW),
                    in_=bass.AP(tensor=sap.tensor, offset=sap.offset,
                                ap=[[GW, 128], [128 * GW, KC], [1, GW]]))

            for i in range(KC):
                nc.sync.dma_start(out=xts[i][:, 0:512],
                                  in_=xT[i * 128:(i + 1) * 128, 0:512])
            wload(wk_t, wkT)
            for i in range(KC):
                nc.sync.dma_start(out=xts[i][:, 512:T],
                                  in_=xT[i * 128:(i + 1) * 128, 512:T])
            wload(wq_t, wqT)
            wload(wv_t, wvT)
            wo_t = [pers.tile([128, C], BF16, tag=f"wo{j}", name=f"wo{j}")
                    for j in range(2)]
            for j in range(2):
                nc.sync.dma_start(out=wo_t[j], in_=woT[j * 128:(j + 1) * 128, :])

            # proj outputs / attention outputs, bf16, feature-major
            qts = [pers.tile([128, T], BF16, tag=f"qT{m}", name=f"qT{m}")
                   for m in range(2)]
            kts = [pers.tile([128, T], BF16, tag=f"kT{m}", name=f"kT{m}")
                   for m in range(2)]
            yts = [pers.tile([128, T], BF16, tag=f"yT{m}", name=f"yT{m}")
                   for m in range(2)]

            # bf16 triangular mask for the diagonal 128x128 strip of
            # P^T: keep (1) where col >= row i.e. q >= k, else 0
            trim = pers.tile([128, 128], BF16, tag="trim", name="trim")
            nc.gpsimd.memset(trim, 1.0)
            nc.gpsimd.affine_select(
                out=trim, in_=trim, compare_op=mybir.AluOpType.is_ge,
                fill=0.0, base=0, pattern=[[1, 128]], channel_multiplier=-1)
            # ones [128, 4] in bf16 for V's ones-columns
            ones4 = pers.tile([128, 4], BF16, tag="ones4", name="ones4")
            for j in range(4):
                nc.scalar.activation(
                    out=ones4[:, j:j + 1],
                    in_=nc.const_aps.tensor(1.0, [128, 1]), func=COPY)
            # ones [1, 64] stationary for the PE partition-broadcast of
            # the softmax reciprocals (out[64,512] = ones.T @ recip_row)
            ones64 = pers.tile([1, 64], BF16, tag="ones64", name="ones64")
            nc.scalar.activation(
                out=ones64, in_=nc.const_aps.tensor(1.0, [1, 64]), func=COPY)

            # V in natural [t, d] layout, 65-wide per head (64 v + one)
            vts = [pers.tile([128, HG * 65], BF16, tag=f"V{tb}", name=f"V{tb}")
                   for tb in range(T // 128)]

            # ---- phase 1: k/q projections for pair0 + all of V ----
            with tc.tile_pool(name="pp1", bufs=4, space="PSUM") as pp1, \
                 tc.tile_pool(name="pp2", bufs=2, space="PSUM") as pp2:
                def proj_block(wt, dest, m, pool):
                    for s in range(NS):
                        ps = pool.tile([128, 512], F32, tag="projps",
                                       name="projps")
                        for k in range(KC):
                            nc.tensor.matmul(
                                ps,
                                wslice(wt, k, m),
                                xts[k][:, s * 512:(s + 1) * 512],
                                start=(k == 0), stop=(k == KC - 1))
                        nc.vector.tensor_copy(
                            out=dest[m][:, s * 512:(s + 1) * 512], in_=ps)

                proj_block(wk_t, kts, 0, pp1)
                proj_block(wq_t, qts, 0, pp1)
                for tb in range(T // 128):
                    vps = pp2.tile([128, GW], F32, tag="vps", name="vps")
                    for k in range(KC):
                        nc.tensor.matmul(
                            vps, xts[k][:, tb * 128:(tb + 1) * 128],
                            wslice(wv_t, k),
                            start=(k == 0), stop=(k == KC - 1))
                    vt = vts[tb]
                    nc.vector.tensor_copy(
                        out=vt.rearrange("p (h c) -> p h c", c=65)[:, :, 0:64],
                        in_=vps.rearrange("p (h c) -> p h c", c=64))
                    nc.vector.tensor_copy(
                        out=vt.rearrange("p (h c) -> p h c", c=65)[:, :, 64],
                        in_=ones4)

            # ---- phase 2: attention (pair-outer), k/q-m1 projections
            # interleaved into pair0, out-projection trailing pair1 ----
            with tc.tile_pool(name="mgs", bufs=2, space="PSUM") as mgs, \
                 tc.tile_pool(name="pvs", bufs=1, space="PSUM") as pvs, \
                 tc.tile_pool(name="ops", bufs=2, space="PSUM") as ops, \
                 tc.tile_pool(name="ptp", bufs=6) as ptp, \
                 tc.tile_pool(name="rp", bufs=4) as rp, \
                 tc.tile_pool(name="ost", bufs=4) as ost:

                # deferred k/q m=1 projection, dribbled 2 matmuls at a
                # time into pair0's attention loop (fills PE slack
                # while Act streams exps)
                proj_q = []       # list of (psum_tile_or_None-marker)
                state = {"ps": None}

                def proj_steps():
                    for wt, dest in ((wk_t, kts), (wq_t, qts)):
                        for s in range(NS):
                            for k in range(KC):
                                yield (wt, dest, s, k)

                proj_iter = proj_steps()

                def emit_proj(n):
                    for _ in range(n):
                        step = next(proj_iter, None)
                        if step is None:
                            return
                        wt, dest, s, k = step
                        if k == 0:
                            state["ps"] = ops.tile([128, 512], F32,
                                                   tag="op", name="op")
                        nc.tensor.matmul(
                            state["ps"], wslice(wt, k, 1),
                            xts[k][:, s * 512:(s + 1) * 512],
                            start=(k == 0), stop=(k == KC - 1))
                        if k == KC - 1:
                            nc.vector.tensor_copy(
                                out=dest[1][:, s * 512:(s + 1) * 512],
                                in_=state["ps"])

                def outproj(s):
                    for m in range(8):
                        op = ops.tile([128, 512], F32, tag="op", name="op")
                        for j in range(2):
                            nc.tensor.matmul(
                                op,
                                wo_t[j][:, m * 128:(m + 1) * 128],
                                yts[j][:, s * 512:(s + 1) * 512],
                                start=(j == 0), stop=(j == 1))
                        ot = ost.tile([128, 512], F32, tag="ot", name="ot")
                        nc.vector.tensor_copy(out=ot, in_=op)
                        nc.sync.dma_start(
                            out=outT[m * 128:(m + 1) * 128,
                                     s * 512:(s + 1) * 512],
                            in_=ot)

                # finalize chains are staged so every op's wait is
                # pre-satisfied when its queue reaches it:
                #   A (own span end):  yv copy (frees the PV banks), then
                #     reciprocal of the sums row -> bf16 (the 3.3us
                #     [1,512] reciprocal blocks only the DVE queue,
                #     which carries nothing attention-critical)
                #   B (next span end, just before the trailing
                #     out-projection): PE broadcast of the reciprocal
                #     row into a PSUM tile (K=1 ones matmul -- no DMA
                #     roundtrip) and the DVE normalize multiply
                pend_b = []
                yv_map = {}

                def fin_a(p, s):
                    out = []
                    for hl in range(2):
                        yv = rp.tile([65, 512], F32, tag=f"yv{p}{hl}",
                                     name=f"yv{p}{hl}")
                        nc.vector.tensor_copy(out=yv, in_=pv_cur[hl][0:65, :])
                        rcb = rp.tile([1, 512], BF16, tag=f"rc{p}{hl}",
                                      name=f"rc{p}{hl}")
                        with nc.allow_low_precision(
                                reason="bf16 1/rowsum feeds a bf16 "
                                       "broadcast matmul; ~0.4% rel is "
                                       "within the 2e-2 budget"):
                            nc.vector.reciprocal(out=rcb, in_=yv[64:65, :])
                        out.append((yv, rcb))
                    return out

                def fin_b(p, s, chains):
                    for hl in range(2):
                        yv, rcb = chains[hl]
                        po = hl * 64
                        rbp = ops.tile([128, 512], F32, tag="op", name="op")
                        nc.tensor.matmul(rbp[0:64, :], ones64, rcb,
                                         start=True, stop=True)
                        nc.vector.tensor_mul(
                            out=yts[p][po:po + 64, s * 512:(s + 1) * 512],
                            in0=yv[0:64, :], in1=rbp[0:64, :])

                for p in range(2):
                    qt, kt = qts[p], kts[p]
                    for s in range(NS):
                        pv_cur = [pvs.tile([65, 512], F32, tag=f"pv{hl}",
                                           name=f"pv{hl}") for hl in range(2)]
                        for ki in range(4 * s + 4):
                            c0 = 128 * (ki - 4 * s) if ki >= 4 * s else 0
                            w = 512 - c0
                            q0 = s * 512 + c0
                            mg = mgs.tile([128, 1024], F32, tag="mg",
                                          name="mg")
                            # paired scores: head 2p rows 0-63 ->
                            # bank A, head 2p+1 rows 64-127 -> bank B;
                            # concurrent via row tiling
                            nc.tensor.matmul(
                                mg[:, c0:512],
                                kt[0:64, ki * 128:(ki + 1) * 128],
                                qt[0:64, q0:(s + 1) * 512],
                                start=True, stop=True)
                            nc.tensor.matmul(
                                mg[:, 512 + c0:1024],
                                kt[64:128, ki * 128:(ki + 1) * 128],
                                qt[64:128, q0:(s + 1) * 512],
                                start=True, stop=True)
                            # one exp over both heads' halves
                            pt = ptp.tile([128, 1024], BF16, tag="pt",
                                          name="pt")
                            mga = bass.AP(
                                tensor=mg.tensor, offset=mg.offset + c0,
                                ap=[list(mg.ap[0]), [512, 2], [1, w]])
                            pta = bass.AP(
                                tensor=pt.tensor, offset=pt.offset + c0,
                                ap=[list(pt.ap[0]), [512, 2], [1, w]])
                            nc.scalar.activation(
                                out=pta, in_=mga, func=EXP, scale=SCALE)
                            if ki >= 4 * s:
                                # causal mask on the diagonal strips
                                # (gpsimd queue carries ONLY these)
                                nc.gpsimd.tensor_mul(
                                    out=pt[:, c0:c0 + 128],
                                    in0=pt[:, c0:c0 + 128], in1=trim)
                                nc.gpsimd.tensor_mul(
                                    out=pt[:, 512 + c0:512 + c0 + 128],
                                    in0=pt[:, 512 + c0:512 + c0 + 128],
                                    in1=trim)
                            for hl in range(2):
                                h = 2 * p + hl
                                nc.tensor.matmul(
                                    pv_cur[hl][:, c0:512],
                                    vts[ki][:, h * 65:(h + 1) * 65],
                                    pt[:, 512 * hl + c0:512 * hl + c0 + w],
                                    start=(ki == 0), stop=(ki == 4 * s + 3))
                            if p == 0:
                                emit_proj(2)
                        # span end: B of the previous span's chains,
                        # then this span's A, then the trailing
                        # out-projection
                        for (pp, ss) in pend_b:
                            fin_b(pp, ss, yv_map[(pp, ss)])
                        pend_b.clear()
                        yv_map[(p, s)] = fin_a(p, s)
                        pend_b.append((p, s))
                        if p == 1 and s > 0:
                            outproj(s - 1)
                    if p == 0:
                        emit_proj(64)   # drain any leftovers
                # tail: flush the last span's chain, then its outproj
                for (pp, ss) in pend_b:
                    fin_b(pp, ss, yv_map[(pp, ss)])
                pend_b.clear()
                outproj(NS - 1)
    nc.compile()
    return nc


_NC_CACHE = None


def _get_nc():
    global _NC_CACHE
    if _NC_CACHE is None:
        _NC_CACHE = build_nc()
    return _NC_CACHE


def make_in_maps(x, wq, wk, wv, wo):
    bf = ml_dtypes.bfloat16
    x = np.asarray(x, dtype=np.float32)
    wq = np.asarray(wq, dtype=np.float32)
    wk = np.asarray(wk, dtype=np.float32)
    wv = np.asarray(wv, dtype=np.float32)
    wo = np.asarray(wo, dtype=np.float32)
    in_maps = []
    for core in range(N_CORES):
        b, g = core // HG, core % HG
        rows = slice(g * GW, (g + 1) * GW)
        in_maps.append({
            "xT": np.ascontiguousarray(x[b].T).astype(bf),
            "wqT": np.ascontiguousarray(wq[rows, :].T).astype(bf),
            "wkT": np.ascontiguousarray(wk[rows, :].T).astype(bf),
            "wvT": np.ascontiguousarray(wv[rows, :].T).astype(bf),
            "woT": np.ascontiguousarray(wo[:, rows].T).astype(bf),
        })
    return in_maps


def run(x, wq, wk, wv, wo, trace=False, tmpdir=None):
    nc = _get_nc()
    in_maps = make_in_maps(x, wq, wk, wv, wo)
    res = run_bass_kernel_spmd(nc, in_maps, core_ids=list(range(N_CORES)),
                               trace=trace, tmpdir=tmpdir)
    out = np.zeros((B, T, C), dtype=np.float32)
    for core in range(N_CORES):
        out[core // HG] += res.results[core]["outT"].T
    return out, res


def kernel(x, wq, wk, wv, wo):
    out, _ = run(x, wq, wk, wv, wo)
    return out
